# revision 2
# baseline (speedup 1.0000x reference)
"""Multi-head attention (B=2, S=2048, D=1024, H=16) on 8 TRN2 NeuronCores.

Sharding: tensor-parallel over heads. Core c owns heads {2c, 2c+1}:
  - Wq/Wk/Wv column-sliced per core; projections computed in transposed
    layout [head_cols, tokens] via PE transposes of q/k/v tiles.
  - Attention (QK^T, softmax, PV) per (batch, local head); softmax weights
    are an output, written per core as [2, 2, 2048, 2048].
  - Output projection: AllToAll re-shards attn_out from head-sharded to
    token-sharded; each core computes its 512 token rows against full Wc.
Host assembles: concat head-blocks for attn_wgts, concat token rows for out.

Matmuls run as float32r (full-rate fp32 mode, ~1e-4 rel err).
"""

import numpy as np

import concourse.bass as bass
import concourse.mybir as mybir
import concourse.tile as tile
from concourse import bacc
from concourse.bass_utils import run_bass_kernel_spmd
from concourse.masks import make_identity

F32 = mybir.dt.float32
F32R = mybir.dt.float32r
AF = mybir.ActivationFunctionType

NCORES = 8
P = 128
B = 2
S = 2048
D = 1024
NT = B * S  # 4096 flattened tokens, batch-major
DH = 64  # head dim
HLOC = 2  # heads per core
TOKB = 512  # phase-1 token block

_CACHE = {}


def _build():
    nc = bacc.Bacc("TRN2", debug=False, num_devices=NCORES)

    q_d = nc.dram_tensor("q", [NT, D], F32, kind="ExternalInput")
    k_d = nc.dram_tensor("k", [NT, D], F32, kind="ExternalInput")
    v_d = nc.dram_tensor("v", [NT, D], F32, kind="ExternalInput")
    wq_d = nc.dram_tensor("wq", [D, P], F32, kind="ExternalInput")
    wk_d = nc.dram_tensor("wk", [D, P], F32, kind="ExternalInput")
    wv_d = nc.dram_tensor("wv", [D, P], F32, kind="ExternalInput")
    wc_d = nc.dram_tensor("wc", [D, D], F32, kind="ExternalInput")
    bq_d = nc.dram_tensor("bq", [P, 1], F32, kind="ExternalInput")
    bk_d = nc.dram_tensor("bk", [P, 1], F32, kind="ExternalInput")
    bv_d = nc.dram_tensor("bv", [P, 1], F32, kind="ExternalInput")
    bc_d = nc.dram_tensor("bc", [D, 1], F32, kind="ExternalInput")

    aw_d = nc.dram_tensor("aw", [B, HLOC, S, S], F32, kind="ExternalOutput")
    out_d = nc.dram_tensor("outp", [NT // NCORES, D], F32, kind="ExternalOutput")

    with tile.TileContext(nc) as tc:
        with (
            tc.tile_pool(name="const", bufs=1) as const,
            tc.tile_pool(name="persist", bufs=1) as persist,
            tc.tile_pool(name="dram", bufs=1, space="DRAM") as dram,
        ):
            # ---------- constants ----------
            ident = const.tile([P, P], F32)
            make_identity(nc, ident[:])
            ident_r = const.tile([P, P], F32R)
            nc.vector.tensor_copy(ident_r[:], ident[:])

            bq_sb = const.tile([P, 1], F32)
            bk_sb = const.tile([P, 1], F32)
            bv_sb = const.tile([P, 1], F32)
            bc_sb = const.tile([P, 8], F32)
            nc.sync.dma_start(bq_sb[:], bq_d[:])
            nc.sync.dma_start(bk_sb[:], bk_d[:])
            nc.sync.dma_start(bv_sb[:], bv_d[:])
            nc.sync.dma_start(
                bc_sb[:].rearrange("p (c o) -> p c o", o=1),
                bc_d[:].rearrange("(c p) o -> p c o", p=P),
            )

            # weights, rounded to fp32r
            wq_r = const.tile([P, 1024], F32R)
            wk_r = const.tile([P, 1024], F32R)
            wv_r = const.tile([P, 1024], F32R)
            wc_r = const.tile([P, 8192], F32R)

            with tc.tile_pool(name="wstage", bufs=2) as wstage:
                for w_d, w_r in ((wq_d, wq_r), (wk_d, wk_r), (wv_d, wv_r)):
                    st = wstage.tile([P, 1024], F32, tag="wst")
                    nc.sync.dma_start(
                        st[:].rearrange("p (f m) -> p f m", f=8),
                        w_d[:].rearrange("(f p) m -> p f m", p=P),
                    )
                    nc.vector.tensor_copy(w_r[:], st[:])
                for ft in range(8):
                    st = wstage.tile([P, 1024], F32, tag="wst")
                    nc.sync.dma_start(st[:], wc_d[ft * P : (ft + 1) * P, :])
                    nc.vector.tensor_copy(
                        wc_r[:, ft * 1024 : (ft + 1) * 1024], st[:]
                    )

            # ---------- persistent activations ----------
            qhT = persist.tile([P, NT], F32R)  # [2h x 64d, tokens]
            khT = persist.tile([P, NT], F32R)
            vh = persist.tile([P, NT], F32R)  # [key%128, ktile*128 + col]
            aoT = persist.tile([P, NT], F32)  # attn_out^T

            # ---------- phase 1: projections ----------
            with (
                tc.tile_pool(name="nat", bufs=2) as natp,
                tc.tile_pool(name="qT", bufs=2) as qTp,
                tc.tile_pool(name="vtmp", bufs=2) as vtmpp,
                tc.tile_pool(name="tp_ps", bufs=3, space="PSUM") as tpp,
                tc.tile_pool(name="proj_ps", bufs=2, space="PSUM") as projp,
            ):
                eng_i = 0
                for tb in range(NT // TOKB):
                    t0 = tb * TOKB
                    for src_d, w_r, b_sb, kind in (
                        (q_d, wq_r, bq_sb, "q"),
                        (k_d, wk_r, bk_sb, "k"),
                        (v_d, wv_r, bv_sb, "v"),
                    ):
                        nat = natp.tile([P, 4096], F32, tag="nat")
                        nc.sync.dma_start(
                            nat[:].rearrange("p (s f) -> p s f", s=4),
                            src_d[t0 : t0 + TOKB, :].rearrange(
                                "(s p) f -> p s f", p=P
                            ),
                        )
                        qT = qTp.tile([P, 4096], F32R, tag="qT")
                        qT8 = qT[:].rearrange("p (f w) -> p f w", f=8)
                        for s in range(4):
                            for fg in range(2):
                                tp = tpp.tile([P, 512], F32, tag="tp")
                                for i in range(4):
                                    ft = fg * 4 + i
                                    nc.tensor.transpose(
                                        tp[:, i * P : (i + 1) * P],
                                        nat[:, s * 1024 + ft * P : s * 1024 + (ft + 1) * P],
                                        ident[:],
                                    )
                                dst = qT8[:, fg * 4 : (fg + 1) * 4, s * P : (s + 1) * P]
                                src = tp[:].rearrange("p (i w) -> p i w", i=4)
                                if eng_i % 2 == 0:
                                    nc.scalar.activation(dst, src, AF.Copy)
                                else:
                                    nc.vector.tensor_copy(dst, src)
                                eng_i += 1
                        pps = projp.tile([P, 512], F32, tag="pp")
                        for ft in range(8):
                            nc.tensor.matmul(
                                pps[:],
                                w_r[:, ft * P : (ft + 1) * P],
                                qT[:, ft * 512 : (ft + 1) * 512],
                                start=(ft == 0),
                                stop=(ft == 7),
                            )
                        if kind == "q":
                            nc.vector.tensor_scalar_add(
                                qhT[:, t0 : t0 + TOKB], pps[:], b_sb[:]
                            )
                        elif kind == "k":
                            nc.vector.tensor_scalar_add(
                                khT[:, t0 : t0 + TOKB], pps[:], b_sb[:]
                            )
                        else:
                            vt = vtmpp.tile([P, 512], F32, tag="vt")
                            nc.vector.tensor_scalar_add(vt[:], pps[:], b_sb[:])
                            tp2 = tpp.tile([P, 512], F32, tag="tp")
                            for s in range(4):
                                nc.tensor.transpose(
                                    tp2[:, s * P : (s + 1) * P],
                                    vt[:, s * P : (s + 1) * P],
                                    ident[:],
                                )
                            nc.scalar.activation(
                                vh[:, t0 : t0 + TOKB], tp2[:], AF.Copy
                            )

            # ---------- phase 2: attention ----------
            with (
                tc.tile_pool(name="wp", bufs=2) as wpp,
                tc.tile_pool(name="wn", bufs=2) as wnp,
                tc.tile_pool(name="slab", bufs=1) as slabp,
                tc.tile_pool(name="dsm", bufs=4) as dsmp,
                tc.tile_pool(name="l_ps", bufs=2, space="PSUM") as lpp,
                tc.tile_pool(name="tpw_ps", bufs=2, space="PSUM") as tpwp,
                tc.tile_pool(name="pv_ps", bufs=2, space="PSUM") as pvp,
            ):
                slabs = [
                    slabp.tile([P, 4096], F32R, tag=f"slab{j}", name=f"slab{j}")
                    for j in range(HLOC)
                ]
                eng_i = 0
                for b in range(B):
                    kb = b * S
                    for qc in range(8):  # 256-query groups
                        for j in range(HLOC):
                            slab16 = slabs[j][:].rearrange(
                                "p (t w) -> p t w", t=16
                            )
                            for qh_i in range(2):
                                qb = qc * 2 + qh_i
                                t0 = b * S + qb * P
                                wp = wpp.tile([P, S], F32, tag="wp")
                                dacc = dsmp.tile([P, 4], F32, tag="ds")
                                for ch in range(2):
                                    lps = lpp.tile([P, 1024], F32, tag="l")
                                    for hf in range(2):
                                        k0 = kb + ch * 1024 + hf * 512
                                        nc.tensor.matmul(
                                            lps[:, hf * 512 : (hf + 1) * 512],
                                            qhT[
                                                j * DH : (j + 1) * DH,
                                                t0 : t0 + P,
                                            ],
                                            khT[
                                                j * DH : (j + 1) * DH,
                                                k0 : k0 + 512,
                                            ],
                                            start=True,
                                            stop=True,
                                        )
                                    nc.scalar.activation(
                                        wp[:, ch * 1024 : (ch + 1) * 1024],
                                        lps[:],
                                        AF.Exp,
                                        scale=0.125,
                                        accum_out=dacc[:, ch : ch + 1],
                                    )
                                dsum = dsmp.tile([P, 1], F32, tag="dsum")
                                nc.vector.tensor_add(
                                    dsum[:], dacc[:, 0:1], dacc[:, 1:2]
                                )
                                rec = dsmp.tile([P, 1], F32, tag="rec")
                                nc.vector.reciprocal(rec[:], dsum[:])
                                wn = wnp.tile([P, S], F32R, tag="wn")
                                nc.vector.tensor_scalar_mul(wn[:], wp[:], rec[:])
                                nc.sync.dma_start(
                                    aw_d[b, j, qb * P : (qb + 1) * P, :],
                                    wn[:].bitcast(F32),
                                )
                                for kt4 in range(4):
                                    tpw = tpwp.tile([P, 512], F32R, tag="tpw")
                                    for i in range(4):
                                        kt = kt4 * 4 + i
                                        nc.tensor.transpose(
                                            tpw[:, i * P : (i + 1) * P],
                                            wn[:, kt * P : (kt + 1) * P],
                                            ident_r[:],
                                        )
                                    dst = slab16[
                                        :,
                                        kt4 * 4 : (kt4 + 1) * 4,
                                        qh_i * P : (qh_i + 1) * P,
                                    ]
                                    src = tpw[:].rearrange(
                                        "p (i w) -> p i w", i=4
                                    )
                                    if eng_i % 2 == 0:
                                        nc.scalar.activation(dst, src, AF.Copy)
                                    else:
                                        nc.vector.tensor_copy(dst, src)
                                    eng_i += 1
                            # PV for this 256-query group, head j
                            pvps = pvp.tile([DH, 256], F32, tag="pv")
                            for kt in range(16):
                                ktg = 16 * b + kt
                                nc.tensor.matmul(
                                    pvps[:],
                                    vh[
                                        :,
                                        ktg * P + j * DH : ktg * P + (j + 1) * DH,
                                    ],
                                    slabs[j][:, kt * 256 : (kt + 1) * 256],
                                    start=(kt == 0),
                                    stop=(kt == 15),
                                )
                            nc.vector.tensor_copy(
                                aoT[
                                    j * DH : (j + 1) * DH,
                                    b * S + qc * 256 : b * S + (qc + 1) * 256,
                                ],
                                pvps[:],
                            )

            # ---------- phase 3: all-to-all + output projection ----------
            a2a_in = dram.tile([NCORES, P, 512], F32)
            a2a_out = dram.tile([NCORES, P, 512], F32)
            for jj in range(NCORES):
                nc.sync.dma_start(
                    a2a_in[jj], aoT[:, jj * 512 : (jj + 1) * 512]
                )
            nc.gpsimd.collective_compute(
                "AllToAll",
                mybir.AluOpType.bypass,
                replica_groups=[list(range(NCORES))],
                ins=[a2a_in[:].opt()],
                outs=[a2a_out[:].opt()],
            )
            with (
                tc.tile_pool(name="cst", bufs=2) as cstp,
                tc.tile_pool(name="oout", bufs=2) as ooutp,
                tc.tile_pool(name="o_ps", bufs=2, space="PSUM") as opsp,
                tc.tile_pool(name="tpo_ps", bufs=2, space="PSUM") as tpop,
            ):
                crs = persist.tile([P, 4096], F32R)
                for ft in range(8):
                    cst = cstp.tile([P, 512], F32, tag="cst")
                    nc.sync.dma_start(cst[:], a2a_out[ft])
                    nc.vector.tensor_copy(
                        crs[:, ft * 512 : (ft + 1) * 512], cst[:]
                    )
                for ct in range(8):
                    ops = opsp.tile([P, 512], F32, tag="ops")
                    for ft in range(8):
                        nc.tensor.matmul(
                            ops[:],
                            wc_r[:, (ft * 8 + ct) * P : (ft * 8 + ct + 1) * P],
                            crs[:, ft * 512 : (ft + 1) * 512],
                            start=(ft == 0),
                            stop=(ft == 7),
                        )
                    otmp = ooutp.tile([P, 512], F32, tag="otmp")
                    nc.vector.tensor_scalar_add(
                        otmp[:], ops[:], bc_sb[:, ct : ct + 1]
                    )
                    tpo = tpop.tile([P, 512], F32, tag="tpo")
                    for s in range(4):
                        nc.tensor.transpose(
                            tpo[:, s * P : (s + 1) * P],
                            otmp[:, s * P : (s + 1) * P],
                            ident[:],
                        )
                    onat = ooutp.tile([P, 512], F32, tag="onat")
                    nc.scalar.activation(onat[:], tpo[:], AF.Copy)
                    nc.sync.dma_start(
                        out_d[:]
                        .rearrange("(s p) m -> p s m", p=P)[
                            :, :, ct * P : (ct + 1) * P
                        ],
                        onat[:].rearrange("p (s c) -> p s c", s=4),
                    )

    nc.compile()
    return nc


def _get_nc():
    if "nc" not in _CACHE:
        _CACHE["nc"] = _build()
    return _CACHE["nc"]


def _make_in_maps(q, k, v, Wq, bq, Wk, bk, Wv, bv, Wc, bc):
    qf = np.ascontiguousarray(np.asarray(q, dtype=np.float32).reshape(NT, D))
    kf = np.ascontiguousarray(np.asarray(k, dtype=np.float32).reshape(NT, D))
    vf = np.ascontiguousarray(np.asarray(v, dtype=np.float32).reshape(NT, D))
    Wq = np.asarray(Wq, dtype=np.float32)
    Wk = np.asarray(Wk, dtype=np.float32)
    Wv = np.asarray(Wv, dtype=np.float32)
    Wc = np.ascontiguousarray(np.asarray(Wc, dtype=np.float32))
    bq = np.asarray(bq, dtype=np.float32).reshape(D, 1)
    bk = np.asarray(bk, dtype=np.float32).reshape(D, 1)
    bv = np.asarray(bv, dtype=np.float32).reshape(D, 1)
    bc = np.ascontiguousarray(np.asarray(bc, dtype=np.float32).reshape(D, 1))
    in_maps = []
    for c in range(NCORES):
        sl = slice(c * P, (c + 1) * P)
        in_maps.append(
            {
                "q": qf,
                "k": kf,
                "v": vf,
                "wq": np.ascontiguousarray(Wq[:, sl]),
                "wk": np.ascontiguousarray(Wk[:, sl]),
                "wv": np.ascontiguousarray(Wv[:, sl]),
                "wc": Wc,
                "bq": np.ascontiguousarray(bq[sl]),
                "bk": np.ascontiguousarray(bk[sl]),
                "bv": np.ascontiguousarray(bv[sl]),
                "bc": bc,
            }
        )
    return in_maps


def _assemble(results):
    H = NCORES * HLOC
    attn = np.empty((B, H, S, S), dtype=np.float32)
    out = np.empty((NT, D), dtype=np.float32)
    for c in range(NCORES):
        aw = results[c]["aw"]
        for j in range(HLOC):
            attn[:, HLOC * c + j] = aw[:, j]
        out[c * 512 : (c + 1) * 512] = results[c]["outp"]
    return out.reshape(B, S, D), attn


def run(q, k, v, Wq, bq, Wk, bk, Wv, bv, Wc, bc, **rk):
    nc = _get_nc()
    in_maps = _make_in_maps(q, k, v, Wq, bq, Wk, bk, Wv, bv, Wc, bc)
    res = run_bass_kernel_spmd(nc, in_maps, core_ids=list(range(NCORES)), **rk)
    out, attn = _assemble(res.results)
    return (out, attn), res


def kernel(q, k, v, Wq, bq, Wk, bk, Wv, bv, Wc, bc):
    (out, attn), _ = run(q, k, v, Wq, bq, Wk, bk, Wv, bv, Wc, bc)
    return out, attn
